# revision 4
# baseline (speedup 1.0000x reference)
"""Trainium2 Bass kernel for nn_CrossAttention (gram-softmax-attention).

Per-sample computation (B=8 samples, data-parallel, one per NeuronCore):
    S = src[b]  [C=512, N=4096]   (flattened HW)
    D = dst[b]  [C=512, N=4096]
    A = S @ S.T                   [512, 512]  (symmetric gram matrix)
    P = softmax(A, axis=0)        (column softmax, torch dim=1 semantics)
    out[b, i, n] = sum_j P[i, j] D[j, n]

Structure notes:
  * A is symmetric, so the row-softmax of the stored [i, j] gram tile equals
    P[j, i] laid out as [j (partition), i (free)] -- exactly the lhsT
    (stationary operand) layout the second matmul needs.  Only one transpose
    (S -> S^T) is required, done on the TensorEngine in 128x128 blocks.
  * Precision: the matmuls run fp8e4 with perf_mode=DoubleRow (2 contraction
    rows per PE cell, ~1.5x bf16 throughput).  Output accuracy is protected
    by restructuring the second matmul as
        out = D + (P - I) @ D
    The correction operand P - I is the softmax deviation from identity
    (tiny for this problem's gram margins), so fp8 on the correction path
    perturbs the result far below the bf16 output quantization.
  * fp8 in the gram matmul is harmless: the softmax column margins are
    O(|S_j|^2) ~ 4096 vs off-diagonal noise ~ O(300), while fp8 gram error
    is O(10); exp underflow keeps the softmax saturated either way.
  * HBM traffic is the roofline (358 GB/s/core): src+dst reads are fp32
    (16.8 MB), the output is written bf16 (4.2 MB) and upcast on the host,
    for ~59 us/core of mandatory traffic.  dst is cast to bf16 in-flight
    (SWDGE) -- the +D re-add then quantizes exactly like the bf16 store, so
    nothing is lost.  All fp8 copies ride DVE drains that exist anyway.
"""

import numpy as np

import concourse.bass as bass
import concourse.mybir as mybir
import concourse.tile as tile
from concourse import bacc, bass_utils
from concourse.bass import ds, ts
from concourse.masks import make_identity

# Problem shape (hardcoded per spec)
B = 8
C = 512
H = W = 64
N = H * W  # 4096
N_CORES = 8
P = 128

MT = C // P      # 4 row tiles of the gram matrix
KC = N // P      # 32 contraction chunks for the gram matmul
KP = KC // 2     # 16 DoubleRow pair-chunks for the gram matmul
KJ = C // P      # 4 contraction chunks for the second matmul
FD = 512         # matmul moving free dim (one PSUM bank of fp32)
NF = N // FD     # 8 free chunks for the second matmul

PANELS = 4
PW = N // PANELS   # 1024 source panel width
KPP = PW // P      # 8 transpose chunks per panel

F32 = mybir.dt.float32
BF16 = mybir.dt.bfloat16
FP8 = mybir.dt.float8e4
AX = mybir.AxisListType
AF = mybir.ActivationFunctionType
DR = mybir.MatmulPerfMode.DoubleRow

_CACHE = {}


def _emit(tc, nc, src, dst, out):
    with (
        tc.tile_pool(name="consts", bufs=1) as consts,
        tc.tile_pool(name="spool", bufs=2) as spool,
        tc.tile_pool(name="stpool", bufs=1) as stpool,
        tc.tile_pool(name="dpool", bufs=1) as dpool,
        tc.tile_pool(name="rpool", bufs=2) as rpool,
        tc.tile_pool(name="stats", bufs=4) as stats,
        tc.tile_pool(name="opool", bufs=4) as opool,
        tc.tile_pool(name="pa", bufs=4, space="PSUM") as pa_pool,
        tc.tile_pool(name="pt", bufs=2, space="PSUM") as pt_pool,
        tc.tile_pool(name="po", bufs=2, space="PSUM") as po_pool,
    ):
        ident_b = consts.tile([P, P], BF16, name="ident_b")
        make_identity(nc, ident_b)
        ident_f = consts.tile([P, P], F32, name="ident_f")
        make_identity(nc, ident_f)

        # S^T in fp8: [n mod 128, n_chunk, i]  (16 KiB/partition)
        St = stpool.tile([P, KC, C], FP8, name="St")
        # dst resident bf16 (final add) + fp8 (correction matmul)
        D = dpool.tile([P, KJ, N], BF16, name="D")
        Db = dpool.tile([P, KJ, N], FP8, name="Db")
        # row-softmaxed gram minus I, fp8, in lhsT layout
        Rb = rpool.tile([P, KJ, C], FP8, name="Rb")

        # Gram accumulators A[128*mt + ., :] -- one PSUM bank each.
        psA = [pa_pool.tile([P, C], F32, tag="pa", name=f"psA{mt}") for mt in range(MT)]

        def gram_pair(kp):
            for mt in range(MT):
                nc.tensor.matmul(
                    psA[mt],
                    lhsT=St[:, ds(2 * kp, 2), ts(mt, P)],
                    rhs=St[:, ds(2 * kp, 2), :],
                    start=(kp == 0),
                    stop=(kp == KP - 1),
                    perf_mode=DR,
                )

        # All input loads ride one SWDGE queue so ordering is strict: src
        # panels first (they gate the transpose->gram->softmax chain), dst
        # chunks behind them.  Each panel is ONE 2 MiB-read DMA via a 3D
        # access pattern [p, row-block, col]; the in-flight cast halves the
        # SBUF-side traffic.
        src_3d = src.rearrange("(mt p) n -> p mt n", p=P)
        panel_tiles = []
        for p in range(PANELS):
            s = spool.tile([P, MT, PW], BF16, tag="s", name=f"s_{p}")
            nc.gpsimd.dma_start(s, src_3d[:, :, ts(p, PW)])
            panel_tiles.append(s)
        for kj in range(KJ):
            nc.gpsimd.dma_start(D[:, kj, :], dst[ts(kj, P), :])

        # Phase 1+2 (pipelined): PE-transpose bf16 S panels, DVE-drain them
        # into St casting to fp8, with the fp8 DoubleRow gram running one
        # pair-chunk behind the drains.
        for p in range(PANELS):
            s_panel = panel_tiles[p]
            for kk in range(KPP):
                k = p * KPP + kk
                pt = pt_pool.tile([P, C], BF16, tag="pt", name=f"pt{k}")
                for mt in range(MT):
                    nc.tensor.transpose(
                        pt[:, ts(mt, P)], s_panel[:, mt, ts(kk, P)], ident_b
                    )
                nc.vector.tensor_copy(out=St[:, k, :], in_=pt[:])
                if k % 2 == 1 and k >= 3:
                    gram_pair((k - 1) // 2 - 1)
        gram_pair(KP - 1)

        # Softmax along the free axis of each stored gram tile (== reference's
        # column softmax by symmetry), already in the [j (part), i (free)]
        # lhsT layout.  Subtract I in fp32, then cast to fp8: Rb = P - I.
        for mt in range(MT):
            negmax = stats.tile([P, 1], F32, tag="negmax", name=f"negmax{mt}")
            sumexp = stats.tile([P, 1], F32, tag="sumexp", name=f"sumexp{mt}")
            rec = stats.tile([P, 1], F32, tag="rec", name=f"rec{mt}")
            R = rpool.tile([P, C], F32, tag="r", name=f"R{mt}")
            nc.vector.reduce_max(negmax, psA[mt], axis=AX.X, negate=True)
            nc.scalar.activation(
                R, psA[mt], AF.Exp,
                bias=negmax, scale=1.0, accum_out=sumexp,
            )
            nc.vector.reciprocal(rec, sumexp)
            nc.vector.tensor_scalar_mul(R, R, rec)
            nc.vector.tensor_tensor(
                R[:, ds(mt * P, P)],
                R[:, ds(mt * P, P)],
                ident_f,
                mybir.AluOpType.subtract,
            )
            nc.vector.tensor_copy(out=Rb[:, mt, :], in_=R)

        # fp8 casts of the dst chunks, in DVE program order matching their
        # DMA arrival times so the queue never head-blocks.
        for kj in range(KJ):
            nc.vector.tensor_copy(out=Db[:, kj, :], in_=D[:, kj, :])

        # Correction matmul (fp8 DoubleRow) + re-add of D:
        #   out[i, n] = D[i, n] + sum_j (P - I)[i, j] D[j, n]
        # Split into two passes accumulating through SBUF so pass A (kj 0-1)
        # overlaps the tail of the dst load; only pass B (kj 2-3) serializes
        # after the last dst chunk.  Pass A's accumulators recycle the gram's
        # PSUM banks (free once the softmax drained them), giving a 4-deep
        # PE->DVE pipeline there.
        otiles = [opool.tile([P, N], BF16, tag="o", name=f"o{mt}") for mt in range(MT)]
        for mt in range(MT):
            for nf in range(NF):
                po = pa_pool.tile([P, FD], F32, tag="pa", name=f"poA{mt}_{nf}")
                nc.tensor.matmul(
                    po,
                    lhsT=Rb[:, ds(0, 2), ts(mt, P)],
                    rhs=Db[:, ds(0, 2), ts(nf, FD)],
                    start=True,
                    stop=True,
                    perf_mode=DR,
                )
                nc.vector.tensor_tensor(
                    otiles[mt][:, ts(nf, FD)],
                    po[:],
                    D[:, mt, ts(nf, FD)],
                    mybir.AluOpType.add,
                )
        for mt in range(MT):
            otile = otiles[mt]
            for nf in range(NF):
                po = po_pool.tile([P, FD], F32, tag="po", name=f"poB{mt}_{nf}")
                nc.tensor.matmul(
                    po,
                    lhsT=Rb[:, ds(2, 2), ts(mt, P)],
                    rhs=Db[:, ds(2, 2), ts(nf, FD)],
                    start=True,
                    stop=True,
                    perf_mode=DR,
                )
                nc.vector.tensor_tensor(
                    otile[:, ts(nf, FD)],
                    po[:],
                    otile[:, ts(nf, FD)],
                    mybir.AluOpType.add,
                )
                if nf == NF // 2 - 1:
                    nc.sync.dma_start(
                        out[ts(mt, P), ds(0, N // 2)], otile[:, : N // 2]
                    )
            nc.sync.dma_start(
                out[ts(mt, P), ds(N // 2, N // 2)], otile[:, N // 2 :]
            )


def _build(reps=1):
    nc = bacc.Bacc(
        "TRN2",
        target_bir_lowering=False,
        debug=False,
        enable_asserts=False,
        num_devices=N_CORES,
    )
    src = nc.dram_tensor("src", (C, N), F32, kind="ExternalInput").ap()
    dst = nc.dram_tensor("dst", (C, N), F32, kind="ExternalInput").ap()
    out = nc.dram_tensor("out", (C, N), BF16, kind="ExternalOutput").ap()
    with tile.TileContext(nc) as tc:
        for _ in range(reps):
            _emit(tc, nc, src, dst, out)
    nc.compile()
    return nc


def _build_looped(loop_n):
    """Bench-only variant: the kernel body inside a hardware For_i loop, so
    one NEFF execution runs it loop_n times (amplifies device time far above
    the per-call dispatch noise of the axon relay)."""
    nc = bacc.Bacc(
        "TRN2",
        target_bir_lowering=False,
        debug=False,
        enable_asserts=False,
        num_devices=N_CORES,
    )
    src = nc.dram_tensor("src", (C, N), F32, kind="ExternalInput").ap()
    dst = nc.dram_tensor("dst", (C, N), F32, kind="ExternalInput").ap()
    out = nc.dram_tensor("out", (C, N), BF16, kind="ExternalOutput").ap()
    with tile.TileContext(nc) as tc:
        with tc.For_i(0, loop_n, 1, hint_engines=(mybir.EngineType.PE,)):
            _emit(tc, nc, src, dst, out)
    nc.compile()
    return nc


def get_nc():
    if "nc" not in _CACHE:
        _CACHE["nc"] = _build()
    return _CACHE["nc"]


def _in_maps(src_features, dst_features):
    src = np.ascontiguousarray(
        np.asarray(src_features, dtype=np.float32).reshape(B, C, N)
    )
    dst = np.ascontiguousarray(
        np.asarray(dst_features, dtype=np.float32).reshape(B, C, N)
    )
    return [{"src": src[b], "dst": dst[b]} for b in range(B)]


def kernel_with_results(src_features, dst_features, trace=False):
    nc = get_nc()
    res = bass_utils.run_bass_kernel_spmd(
        nc,
        _in_maps(src_features, dst_features),
        core_ids=list(range(N_CORES)),
        trace=trace,
    )
    out = np.stack(
        [np.asarray(res.results[b]["out"]).astype(np.float32) for b in range(B)]
    )
    return out.reshape(B, C, H, W), res


def kernel(src_features, dst_features):
    out, _ = kernel_with_results(src_features, dst_features)
    return out


def _make_runner(nc):
    """jit'd runner for a prebuilt nc: (src, dst, zeros) device arrays ->
    out device array.  Mirrors run_bass_via_pjrt's multi-core path but
    without donation or per-call host transfers."""
    import jax
    import jax.numpy as jnp
    from jax.sharding import Mesh, PartitionSpec
    from jax.experimental.shard_map import shard_map

    from concourse import bass2jax
    from concourse.bass2jax import _bass_exec_p, partition_id_tensor

    bass2jax.install_neuronx_cc_hook()

    in_names = ["src", "dst", "out"]
    if nc.partition_id_tensor is not None:
        in_names.append(nc.partition_id_tensor.name)
    out_avals = [jax.core.ShapedArray((C, N), jnp.bfloat16)]

    def _body(s, d, z):
        operands = [s, d, z]
        if nc.partition_id_tensor is not None:
            operands.append(partition_id_tensor())
        outs = _bass_exec_p.bind(
            *operands,
            out_avals=tuple(out_avals),
            in_names=tuple(in_names),
            out_names=("out",),
            lowering_input_output_aliases=(),
            sim_require_finite=True,
            sim_require_nnan=True,
            nc=nc,
        )
        return tuple(outs)

    devices = jax.devices()[:N_CORES]
    mesh = Mesh(np.asarray(devices), ("core",))
    return jax.jit(
        shard_map(
            _body, mesh=mesh,
            in_specs=(PartitionSpec("core"),) * 3,
            out_specs=(PartitionSpec("core"),),
            check_rep=False,
        ),
        donate_argnums=(2,),
        keep_unused=True,
    )


def bench(src_features, dst_features, iters=12, warmup=3,
          loop_lo=16, loop_hi=128):
    """Measure per-kernel execution time by differencing two For_i-looped
    NEFFs (loop_hi vs loop_lo iterations of the body in one execution); the
    axon dispatch round-trip and NEFF-load overheads cancel in the
    difference.  Returns (per_iter_ns, out_np)."""
    import time

    import jax
    import jax.numpy as jnp
    import ml_dtypes
    from jax.sharding import Mesh, NamedSharding, PartitionSpec

    src = np.ascontiguousarray(
        np.asarray(src_features, np.float32).reshape(B * C, N))
    dst = np.ascontiguousarray(
        np.asarray(dst_features, np.float32).reshape(B * C, N))
    zeros = np.zeros((B * C, N), ml_dtypes.bfloat16)
    mesh = Mesh(np.asarray(jax.devices()[:N_CORES]), ("core",))
    sh = NamedSharding(mesh, PartitionSpec("core"))
    s_dev = jax.device_put(src, sh)
    d_dev = jax.device_put(dst, sh)

    def time_f(f, label):
        # The out operand is donated (the NEFF writes into that buffer), so
        # chain each call's output in as the next call's out operand.
        z = jax.device_put(zeros, sh)
        for _ in range(warmup):
            (z,) = f(s_dev, d_dev, z)
            z.block_until_ready()
        ts = []
        for _ in range(iters):
            t0 = time.perf_counter()
            (z,) = f(s_dev, d_dev, z)
            z.block_until_ready()
            ts.append(time.perf_counter() - t0)
        a = np.asarray(ts) * 1e3
        print(f"  [{label}] med={np.median(a):.3f} p10={np.percentile(a,10):.3f} "
              f"p90={np.percentile(a,90):.3f} min={a.min():.3f} ms")
        return float(np.median(ts)), z

    key_lo, key_hi = f"nc_loop{loop_lo}", f"nc_loop{loop_hi}"
    if key_lo not in _CACHE:
        _CACHE[key_lo] = _build_looped(loop_lo)
    if key_hi not in _CACHE:
        _CACHE[key_hi] = _build_looped(loop_hi)
    flo = _make_runner(_CACHE[key_lo])
    fhi = _make_runner(_CACHE[key_hi])

    tlo, olo = time_f(flo, f"loop={loop_lo}")
    thi, ohi = time_f(fhi, f"loop={loop_hi}")
    per_iter_ns = (thi - tlo) / (loop_hi - loop_lo) * 1e9
    print(f"bench: t{loop_lo}={tlo*1e3:.3f} ms  t{loop_hi}={thi*1e3:.3f} ms  "
          f"-> per-kernel {per_iter_ns:.0f} ns")
    out = np.asarray(olo).astype(np.float32).reshape(B, C, H, W)
    return per_iter_ns, out


# revision 9
# speedup vs baseline: 1.1847x; 1.1847x over previous
"""Trainium2 Bass kernel for nn_CrossAttention (gram-softmax-attention).

Per-sample computation (B=8 samples, data-parallel, one per NeuronCore):
    S = src[b]  [C=512, N=4096]   (flattened HW)
    D = dst[b]  [C=512, N=4096]
    A = S @ S.T                   [512, 512]  (symmetric gram matrix)
    P = softmax(A, axis=0)        (column softmax, torch dim=1 semantics)
    out[b, i, n] = sum_j P[i, j] D[j, n]

Structure notes:
  * A is symmetric, so the row-softmax of the stored [i, j] gram tile equals
    P[j, i] laid out as [j (partition), i (free)] -- exactly the lhsT
    (stationary operand) layout the second matmul needs.  Only one transpose
    (S -> S^T) is required, done on the TensorEngine in 128x128 blocks.
  * Precision: the matmuls run fp8e4 with perf_mode=DoubleRow (2 contraction
    rows per PE cell, ~1.5x bf16 throughput).  Output accuracy is protected
    by restructuring the second matmul as
        out = D + (P - I) @ D
    The correction operand P - I is the softmax deviation from identity
    (tiny for this problem's gram margins), so fp8 on the correction path
    perturbs the result far below the bf16 output quantization.
  * fp8 in the gram matmul is harmless: the softmax column margins are
    O(|S_j|^2) ~ 4096 vs off-diagonal noise ~ O(300), while fp8 gram error
    is O(10); exp underflow keeps the softmax saturated either way.
  * HBM traffic is the roofline (358 GB/s/core): src+dst reads are fp32
    (16.8 MB), the output is written bf16 (4.2 MB) and upcast on the host,
    for ~59 us/core of mandatory traffic.  dst is cast to bf16 in-flight
    (SWDGE) -- the +D re-add then quantizes exactly like the bf16 store, so
    nothing is lost.  All fp8 copies ride DVE drains that exist anyway.
"""

import numpy as np

import concourse.bass as bass
import concourse.mybir as mybir
import concourse.tile as tile
from concourse import bacc, bass_utils
from concourse.bass import ds, ts
from concourse.masks import make_identity

# Problem shape (hardcoded per spec)
B = 8
C = 512
H = W = 64
N = H * W  # 4096
N_CORES = 8
P = 128

MT = C // P      # 4 row tiles of the gram matrix
KC = N // P      # 32 contraction chunks for the gram matmul
KP = KC // 2     # 16 DoubleRow pair-chunks for the gram matmul
KJ = C // P      # 4 contraction chunks for the second matmul
FD = 512         # matmul moving free dim (one PSUM bank of fp32)
NF = N // FD     # 8 free chunks for the second matmul

PANELS = 4
PW = N // PANELS   # 1024 source panel width
KPP = PW // P      # 8 transpose chunks per panel

F32 = mybir.dt.float32
BF16 = mybir.dt.bfloat16
FP8 = mybir.dt.float8e4
AX = mybir.AxisListType
AF = mybir.ActivationFunctionType
DR = mybir.MatmulPerfMode.DoubleRow

_CACHE = {}


def _emit(tc, nc, src, dst, out):
    with (
        tc.tile_pool(name="consts", bufs=1) as consts,
        tc.tile_pool(name="spool", bufs=2) as spool,
        tc.tile_pool(name="stpool", bufs=1) as stpool,
        tc.tile_pool(name="dpool", bufs=1) as dpool,
        tc.tile_pool(name="rpool", bufs=2) as rpool,
        tc.tile_pool(name="stats", bufs=4) as stats,
        tc.tile_pool(name="opool", bufs=2) as opool,
        tc.tile_pool(name="pa", bufs=4, space="PSUM") as pa_pool,
        tc.tile_pool(name="pt", bufs=2, space="PSUM") as pt_pool,
        tc.tile_pool(name="po", bufs=2, space="PSUM") as po_pool,
    ):
        ident_b = consts.tile([P, P], BF16, name="ident_b")
        make_identity(nc, ident_b)
        ident_f = consts.tile([P, P], F32, name="ident_f")
        make_identity(nc, ident_f)
        ident_8 = consts.tile([P, P], FP8, name="ident_8")
        make_identity(nc, ident_8)

        # S^T in fp8: [n mod 128, n_chunk, i]  (16 KiB/partition)
        St = stpool.tile([P, KC, C], FP8, name="St")
        # dst resident bf16 (final add) + fp8 (correction matmul)
        D = dpool.tile([P, KJ, N], BF16, name="D")
        Db = dpool.tile([P, KJ, N], FP8, name="Db")
        # row-softmaxed gram minus I, fp8, in lhsT layout
        Rb = rpool.tile([P, KJ, C], FP8, name="Rb")

        # Gram accumulators A[128*mt + ., :] -- one PSUM bank each.
        psA = [pa_pool.tile([P, C], F32, tag="pa", name=f"psA{mt}") for mt in range(MT)]

        def gram_pair(kp):
            for mt in range(MT):
                nc.tensor.matmul(
                    psA[mt],
                    lhsT=St[:, ds(2 * kp, 2), ts(mt, P)],
                    rhs=St[:, ds(2 * kp, 2), :],
                    start=(kp == 0),
                    stop=(kp == KP - 1),
                    perf_mode=DR,
                )

        # All input loads ride one SWDGE queue so ordering is strict: src
        # panels first (they gate the transpose->gram->softmax chain), dst
        # chunks behind them.  Each panel is ONE 2 MiB-read DMA via a 3D
        # access pattern [p, row-block, col]; the in-flight cast halves the
        # SBUF-side traffic.
        src_3d = src.rearrange("(mt p) n -> p mt n", p=P)
        panel_tiles = []
        for p in range(PANELS):
            s = spool.tile([P, MT, PW], BF16, tag="s", name=f"s_{p}")
            nc.gpsimd.dma_start(s, src_3d[:, :, ts(p, PW)])
            panel_tiles.append(s)
        for kj in range(KJ):
            nc.gpsimd.dma_start(D[:, kj, :], dst[ts(kj, P), :])

        # Phase 1+2 (pipelined): PE-transpose bf16 S panels, drain them into
        # St casting to fp8 -- even chunks on the Scalar engine, odd chunks
        # on DVE, so neither engine paces the pipeline alone -- with the fp8
        # DoubleRow gram running one pair-chunk behind the drains.
        for p in range(PANELS):
            s_panel = panel_tiles[p]
            for kk in range(KPP):
                k = p * KPP + kk
                pt = pt_pool.tile([P, C], BF16, tag="pt", name=f"pt{k}")
                for mt in range(MT):
                    nc.tensor.transpose(
                        pt[:, ts(mt, P)], s_panel[:, mt, ts(kk, P)], ident_b
                    )
                if k % 2 == 0:
                    nc.scalar.activation(St[:, k, :], pt[:], AF.Copy)
                else:
                    nc.vector.tensor_copy(out=St[:, k, :], in_=pt[:])
                    gram_pair((k - 1) // 2 - 1) if k >= 3 else None
        gram_pair(KP - 1)

        # Softmax along the free axis of each stored gram tile (== reference's
        # column softmax by symmetry), already in the [j (part), i (free)]
        # lhsT layout.  The normalizing multiply casts straight into fp8 Rb;
        # the diagonal identity is subtracted in fp8 (the diagonal softmax
        # entry survives the cast exactly whenever it is a power of two, in
        # particular the saturated P[j,j] = 1 case).
        for mt in range(MT):
            negmax = stats.tile([P, 1], F32, tag="negmax", name=f"negmax{mt}")
            sumexp = stats.tile([P, 1], F32, tag="sumexp", name=f"sumexp{mt}")
            rec = stats.tile([P, 1], F32, tag="rec", name=f"rec{mt}")
            R = rpool.tile([P, C], F32, tag="r", name=f"R{mt}")
            nc.vector.reduce_max(negmax, psA[mt], axis=AX.X, negate=True)
            nc.scalar.activation(
                R, psA[mt], AF.Exp,
                bias=negmax, scale=1.0, accum_out=sumexp,
            )
            nc.vector.reciprocal(rec, sumexp)
            nc.vector.tensor_scalar_mul(Rb[:, mt, :], R, rec)
            nc.vector.tensor_tensor(
                Rb[:, mt, ds(mt * P, P)],
                Rb[:, mt, ds(mt * P, P)],
                ident_8,
                mybir.AluOpType.subtract,
            )

        # fp8 casts of the dst chunks, in DVE program order matching their
        # DMA arrival times (DVE is in-order: a cast whose DMA hasn't landed
        # head-blocks everything queued behind it, so these sit between the
        # softmax and the MM2 drains).
        for kj in range(KJ):
            nc.vector.tensor_copy(out=Db[:, kj, :], in_=D[:, kj, :])

        # Correction matmul (fp8 DoubleRow, 2 pair-chunks accumulated in
        # PSUM) + re-add of D:
        #   out[i, n] = D[i, n] + sum_j (P - I)[i, j] D[j, n]
        # The DVE drain adds PSUM f32 to D bf16 and stores bf16.
        for mt in range(MT):
            otile = opool.tile([P, N], BF16, tag="o", name=f"o{mt}")
            for nf in range(NF):
                po = po_pool.tile([P, FD], F32, tag="po", name=f"po{mt}_{nf}")
                for t in range(2):
                    nc.tensor.matmul(
                        po,
                        lhsT=Rb[:, ds(2 * t, 2), ts(mt, P)],
                        rhs=Db[:, ds(2 * t, 2), ts(nf, FD)],
                        start=(t == 0),
                        stop=(t == 1),
                        perf_mode=DR,
                    )
                nc.vector.tensor_tensor(
                    otile[:, ts(nf, FD)],
                    po[:],
                    D[:, mt, ts(nf, FD)],
                    mybir.AluOpType.add,
                )
                if nf == NF // 2 - 1:
                    nc.sync.dma_start(
                        out[ts(mt, P), ds(0, N // 2)], otile[:, : N // 2]
                    )
            nc.sync.dma_start(
                out[ts(mt, P), ds(N // 2, N // 2)], otile[:, N // 2 :]
            )


def _build(reps=1):
    nc = bacc.Bacc(
        "TRN2",
        target_bir_lowering=False,
        debug=False,
        enable_asserts=False,
        num_devices=N_CORES,
    )
    src = nc.dram_tensor("src", (C, N), F32, kind="ExternalInput").ap()
    dst = nc.dram_tensor("dst", (C, N), F32, kind="ExternalInput").ap()
    out = nc.dram_tensor("out", (C, N), BF16, kind="ExternalOutput").ap()
    with tile.TileContext(nc) as tc:
        for _ in range(reps):
            _emit(tc, nc, src, dst, out)
    nc.compile()
    return nc


def _build_looped(loop_n):
    """Bench-only variant: the kernel body inside a hardware For_i loop, so
    one NEFF execution runs it loop_n times (amplifies device time far above
    the per-call dispatch noise of the axon relay)."""
    nc = bacc.Bacc(
        "TRN2",
        target_bir_lowering=False,
        debug=False,
        enable_asserts=False,
        num_devices=N_CORES,
    )
    src = nc.dram_tensor("src", (C, N), F32, kind="ExternalInput").ap()
    dst = nc.dram_tensor("dst", (C, N), F32, kind="ExternalInput").ap()
    out = nc.dram_tensor("out", (C, N), BF16, kind="ExternalOutput").ap()
    with tile.TileContext(nc) as tc:
        with tc.For_i(0, loop_n, 1, hint_engines=(mybir.EngineType.PE,)):
            _emit(tc, nc, src, dst, out)
    nc.compile()
    return nc


def get_nc():
    if "nc" not in _CACHE:
        _CACHE["nc"] = _build()
    return _CACHE["nc"]


def _in_maps(src_features, dst_features):
    src = np.ascontiguousarray(
        np.asarray(src_features, dtype=np.float32).reshape(B, C, N)
    )
    dst = np.ascontiguousarray(
        np.asarray(dst_features, dtype=np.float32).reshape(B, C, N)
    )
    return [{"src": src[b], "dst": dst[b]} for b in range(B)]


def kernel_with_results(src_features, dst_features, trace=False):
    nc = get_nc()
    res = bass_utils.run_bass_kernel_spmd(
        nc,
        _in_maps(src_features, dst_features),
        core_ids=list(range(N_CORES)),
        trace=trace,
    )
    out = np.stack(
        [np.asarray(res.results[b]["out"]).astype(np.float32) for b in range(B)]
    )
    return out.reshape(B, C, H, W), res


def kernel(src_features, dst_features):
    out, _ = kernel_with_results(src_features, dst_features)
    return out


def _make_runner(nc):
    """jit'd runner for a prebuilt nc: (src, dst, zeros) device arrays ->
    out device array.  Mirrors run_bass_via_pjrt's multi-core path but
    without donation or per-call host transfers."""
    import jax
    import jax.numpy as jnp
    from jax.sharding import Mesh, PartitionSpec
    from jax.experimental.shard_map import shard_map

    from concourse import bass2jax
    from concourse.bass2jax import _bass_exec_p, partition_id_tensor

    bass2jax.install_neuronx_cc_hook()

    in_names = ["src", "dst", "out"]
    if nc.partition_id_tensor is not None:
        in_names.append(nc.partition_id_tensor.name)
    out_avals = [jax.core.ShapedArray((C, N), jnp.bfloat16)]

    def _body(s, d, z):
        operands = [s, d, z]
        if nc.partition_id_tensor is not None:
            operands.append(partition_id_tensor())
        outs = _bass_exec_p.bind(
            *operands,
            out_avals=tuple(out_avals),
            in_names=tuple(in_names),
            out_names=("out",),
            lowering_input_output_aliases=(),
            sim_require_finite=True,
            sim_require_nnan=True,
            nc=nc,
        )
        return tuple(outs)

    devices = jax.devices()[:N_CORES]
    mesh = Mesh(np.asarray(devices), ("core",))
    return jax.jit(
        shard_map(
            _body, mesh=mesh,
            in_specs=(PartitionSpec("core"),) * 3,
            out_specs=(PartitionSpec("core"),),
            check_rep=False,
        ),
        donate_argnums=(2,),
        keep_unused=True,
    )


def bench(src_features, dst_features, iters=12, warmup=3,
          loop_lo=16, loop_hi=128):
    """Measure per-kernel execution time by differencing two For_i-looped
    NEFFs (loop_hi vs loop_lo iterations of the body in one execution); the
    axon dispatch round-trip and NEFF-load overheads cancel in the
    difference.  Returns (per_iter_ns, out_np)."""
    import time

    import jax
    import jax.numpy as jnp
    import ml_dtypes
    from jax.sharding import Mesh, NamedSharding, PartitionSpec

    src = np.ascontiguousarray(
        np.asarray(src_features, np.float32).reshape(B * C, N))
    dst = np.ascontiguousarray(
        np.asarray(dst_features, np.float32).reshape(B * C, N))
    zeros = np.zeros((B * C, N), ml_dtypes.bfloat16)
    mesh = Mesh(np.asarray(jax.devices()[:N_CORES]), ("core",))
    sh = NamedSharding(mesh, PartitionSpec("core"))
    s_dev = jax.device_put(src, sh)
    d_dev = jax.device_put(dst, sh)

    def time_f(f, label):
        # The out operand is donated (the NEFF writes into that buffer), so
        # chain each call's output in as the next call's out operand.
        z = jax.device_put(zeros, sh)
        for _ in range(warmup):
            (z,) = f(s_dev, d_dev, z)
            z.block_until_ready()
        ts = []
        for _ in range(iters):
            t0 = time.perf_counter()
            (z,) = f(s_dev, d_dev, z)
            z.block_until_ready()
            ts.append(time.perf_counter() - t0)
        a = np.asarray(ts) * 1e3
        print(f"  [{label}] med={np.median(a):.3f} p10={np.percentile(a,10):.3f} "
              f"p90={np.percentile(a,90):.3f} min={a.min():.3f} ms")
        return float(np.median(ts)), z

    key_lo, key_hi = f"nc_loop{loop_lo}", f"nc_loop{loop_hi}"
    if key_lo not in _CACHE:
        _CACHE[key_lo] = _build_looped(loop_lo)
    if key_hi not in _CACHE:
        _CACHE[key_hi] = _build_looped(loop_hi)
    flo = _make_runner(_CACHE[key_lo])
    fhi = _make_runner(_CACHE[key_hi])

    tlo, olo = time_f(flo, f"loop={loop_lo}")
    thi, ohi = time_f(fhi, f"loop={loop_hi}")
    per_iter_ns = (thi - tlo) / (loop_hi - loop_lo) * 1e9
    print(f"bench: t{loop_lo}={tlo*1e3:.3f} ms  t{loop_hi}={thi*1e3:.3f} ms  "
          f"-> per-kernel {per_iter_ns:.0f} ns")
    out = np.asarray(olo).astype(np.float32).reshape(B, C, H, W)
    return per_iter_ns, out
